# revision 19
# baseline (speedup 1.0000x reference)
"""Bidirectional GRU (T=512, B=32, I=H=512) on 8 Trainium2 NeuronCores.

Sharding: core c -> (direction d = c//4, batch slice j = c%4 of 8).
Backward direction is handled by feeding that core a time-reversed input
and un-reversing its output on the host, so all 8 cores run the same
SPMD program.

Per-core device program:
  Phase 1: xg = x @ w_ih^T + biases as one big fp16 GEMM at full PE
           utilization; xg kept SBUF-resident in fp16 (12 MB).
  Phase 2: 512 sequential GRU steps. Weight-stationary matmuls
           (gate-dim on partitions) so the per-step elementwise work is
           (128, 32)-shaped; h state ping-pongs through 4 fixed SBUF
           slots; per-step y written out by DMA.

Numerics: fp16 operands with fp32 PSUM accumulation and fp32 gate math;
measured end-to-end absmax error vs the fp32 reference ~2.3e-3 on
y (|y|max ~4.1), i.e. ~6e-4 scale-relative.
"""

import numpy as np

T, B, I, H = 512, 32, 512, 512
NB = 8          # batch per core
KC = 4          # contraction chunks (512/128)
GC = 12         # gate-dim chunks (1536/128)
N_CORES = 8
UNROLL = 64
STAGE_AHEAD = 3  # xg prefetch distance (steps)
P1_BLOCK = 512  # phase-1 moving-operand block


def build(t_steps=T, unroll=UNROLL):
    import concourse.bacc as bacc
    import concourse.bass as bass
    import concourse.mybir as mybir
    import concourse.tile as tile
    from contextlib import ExitStack

    f16, f32 = mybir.dt.float16, mybir.dt.float32
    TBS = t_steps * NB

    nc = bacc.Bacc("TRN2", target_bir_lowering=False, debug=False,
                   num_devices=N_CORES)

    xT = nc.dram_tensor("xT", [128, KC, TBS], f16, kind="ExternalInput").ap()
    wih_d = nc.dram_tensor("wih", [128, KC, GC, 128], f16, kind="ExternalInput").ap()
    whh_d = nc.dram_tensor("whh", [128, KC, GC, 128], f16, kind="ExternalInput").ap()
    biasg_d = nc.dram_tensor("biasg", [128, GC], f32, kind="ExternalInput").ap()
    ident_d = nc.dram_tensor("ident", [128, 128], f16, kind="ExternalInput").ap()
    bhhc_d = nc.dram_tensor("bhhc", [128, KC, NB], f16, kind="ExternalInput").ap()
    h0t_d = nc.dram_tensor("h0t", [128, KC, NB], f16, kind="ExternalInput").ap()
    y_d = nc.dram_tensor("y", [t_steps, 128, KC, NB], f16, kind="ExternalOutput").ap()

    ADD = mybir.AluOpType.add
    SUB = mybir.AluOpType.subtract
    SIG = mybir.ActivationFunctionType.Sigmoid
    TANH = mybir.ActivationFunctionType.Tanh

    with ExitStack() as ctx:
        tc = ctx.enter_context(tile.TileContext(nc))
        cp = ctx.enter_context(tc.tile_pool(name="const", bufs=1))
        sp = ctx.enter_context(tc.tile_pool(name="state", bufs=1))
        wp = ctx.enter_context(tc.tile_pool(name="work", bufs=2))

        xts = cp.tile([128, KC, TBS], f16)
        wih_sb = cp.tile([128, KC, GC, 128], f16)
        whh_sb = cp.tile([128, KC, GC, 128], f16)
        bias_sb = cp.tile([128, GC], f32)
        ident_sb = cp.tile([128, 128], f16)
        bhhc_sb = cp.tile([128, KC, NB], f16)
        xg_sb = cp.tile([128, GC, TBS + STAGE_AHEAD * NB], f16)

        nc.sync.dma_start(wih_sb[:], wih_d[:])
        nc.sync.dma_start(whh_sb[:], whh_d[:])
        nc.sync.dma_start(bias_sb[:], biasg_d[:])
        nc.sync.dma_start(ident_sb[:], ident_d[:])
        nc.sync.dma_start(bhhc_sb[:], bhhc_d[:])
        nc.sync.dma_start(xts[:], xT[:])

        # 4 fixed h-state slots; step s reads slot s%4, writes (s+1)%4.
        h_slots = [sp.tile([128, KC, NB], f16, tag=f"h{i}", name=f"h{i}")
                   for i in range(4)]
        nc.sync.dma_start(h_slots[0][:], h0t_d[:])
        # 4 fixed xg staging slots; step s consumes slot s%4 (all-static
        # APs inside the loop), refilled STAGE_AHEAD steps in advance.
        xg_slots = [sp.tile([128, GC, NB], f16, tag=f"xgs{i}", name=f"xgs{i}")
                    for i in range(4)]
        # pad region of xg_sb is read by the tail prefetches; zero it
        nc.vector.memset(xg_sb[:, :, TBS:TBS + STAGE_AHEAD * NB], 0.0)

        # ---- Phase 1: xg[g, t*NB+b] = sum_kc wih[kc,g]^T @ x[kc] + bias[g]
        blk = min(P1_BLOCK, TBS)
        with tc.tile_pool(name="ps1", bufs=4, space="PSUM") as ps1:
            for nb in range(TBS // blk):
                lo, hi = nb * blk, (nb + 1) * blk
                for g in range(GC):
                    ps = ps1.tile([128, blk], f32, tag="p1")
                    for kc in range(KC):
                        nc.tensor.matmul(ps[:], wih_sb[:, kc, g, :],
                                         xts[:, kc, lo:hi],
                                         start=(kc == 0), stop=(kc == KC - 1))
                    nc.vector.tensor_scalar_add(xg_sb[:, g, lo:hi], ps[:],
                                                bias_sb[:, g:g + 1])

        # ---- Phase 2: the recurrence
        # Per-gate psum tiles give precise deps; xg for r/z and b_hh_n are
        # accumulated into PSUM by the PE itself (identity / K=1-row
        # matmuls), so the elementwise chain is:
        #   sig(ps_r) -> *ps_n -> +xg_n -> tanh -> (n-h) -> *zp -> +h
        ps2 = ctx.enter_context(tc.tile_pool(name="ps2", bufs=2, space="PSUM"))
        step_idx = [0]

        # prologue: fill the first STAGE_AHEAD xg staging slots (static)
        for i in range(STAGE_AHEAD):
            nc.sync.dma_start(xg_slots[i][:],
                              xg_sb[:, :, i * NB:(i + 1) * NB])

        def body(iv):
            s = step_idx[0]
            h_prev = h_slots[s % 4]
            h_new = h_slots[(s + 1) % 4]
            xg_t = xg_slots[s % 4]
            step_idx[0] += 1

            # prefetch xg for step iv+STAGE_AHEAD into the slot it will use
            nc.sync.dma_start(
                xg_slots[(s + STAGE_AHEAD) % 4][:],
                xg_sb[:, :, bass.ds((iv + STAGE_AHEAD) * NB, NB)])

            ps_r = ps2.tile([128, KC, NB], f32, tag="ps_r")
            ps_z = ps2.tile([128, KC, NB], f32, tag="ps_z")
            ps_n = ps2.tile([128, KC, NB], f32, tag="ps_n")

            # h-independent psum initializers (identity matmuls) go first:
            # they execute during the PE idle window while this step's h is
            # still being computed. start=True writes xg / b_hh_n into the
            # whole tile; the whh matmuls then accumulate on top
            # (has_written-driven on HW; group checks skipped in sim).
            nc.tensor.matmul(ps_r[:], ident_sb[:], xg_t[:, 0:4, :],
                             start=True, stop=False, skip_group_check=True)
            nc.tensor.matmul(ps_n[:], ident_sb[:], bhhc_sb[:],
                             start=True, stop=False, skip_group_check=True)
            nc.tensor.matmul(ps_z[:], ident_sb[:], xg_t[:, 4:8, :],
                             start=True, stop=False, skip_group_check=True)

            def gate_group(psd, g0):
                for i in range(4):
                    g = g0 + i
                    for kc in range(KC):
                        nc.tensor.matmul(psd[:, i, :], whh_sb[:, kc, g, :],
                                         h_prev[:, kc, :],
                                         start=False,
                                         stop=(i == 3 and kc == KC - 1),
                                         skip_group_check=True)

            gate_group(ps_r, 0)    # r ready earliest
            gate_group(ps_n, 8)    # n next (its chain is longest)
            gate_group(ps_z, 4)    # z last

            r = wp.tile([128, KC, NB], f32, tag="r")
            nc.scalar.activation(r[:], ps_r[:], SIG)
            t1 = wp.tile([128, KC, NB], f32, tag="t1")
            nc.vector.tensor_mul(t1[:], r[:], ps_n[:])
            npre = wp.tile([128, KC, NB], f32, tag="npre")
            nc.gpsimd.tensor_tensor(npre[:], t1[:], xg_t[:, 8:12, :], op=ADD)
            n = wp.tile([128, KC, NB], f32, tag="n")
            tanh_inst = nc.scalar.activation(n[:], npre[:], TANH)
            zp = wp.tile([128, KC, NB], f32, tag="zp")
            zp_inst = nc.scalar.activation(zp[:], ps_z[:], SIG, scale=-1.0)
            tile.add_dep_helper(zp_inst.ins, tanh_inst.ins,
                                reason="keep ACT order: tanh before zp")

            s1 = wp.tile([128, KC, NB], f32, tag="s1")
            nc.vector.tensor_tensor(s1[:], n[:], h_prev[:], op=SUB)
            m = wp.tile([128, KC, NB], f32, tag="m")
            nc.vector.tensor_mul(m[:], zp[:], s1[:])
            nc.vector.tensor_tensor(h_new[:], h_prev[:], m[:], op=ADD)

            nc.sync.dma_start(y_d[bass.ds(iv, 1), :, :, :], h_new[:])

        import concourse.mybir as _mybir

        def unrollable_body(iv0, n_unroll):
            for i in range(n_unroll):
                body(iv0 + i)

        tc.For_i_unrolled_general(0, t_steps, 1, unrollable_body, unroll,
                                  hint_engines=(_mybir.EngineType.PE,))

    nc.compile()
    return nc


def _prep_core(x, h0, w_ih, w_hh, b_ih, b_hh, d, j, t_steps=T):
    bsl = slice(j * NB, (j + 1) * NB)
    xd = x if d == 0 else x[::-1]
    xs = np.ascontiguousarray(xd[:, bsl, :])                     # (T, NB, I)
    xT_ = xs.reshape(t_steps, NB, KC, 128).transpose(3, 2, 0, 1)
    xT_ = np.ascontiguousarray(xT_).reshape(128, KC, t_steps * NB)
    wih = w_ih[d].reshape(GC, 128, KC, 128).transpose(3, 2, 0, 1)
    whh = w_hh[d].reshape(GC, 128, KC, 128).transpose(3, 2, 0, 1)
    bb = b_ih[d].copy()
    bb[:2 * H] += b_hh[d][:2 * H]
    biasg = bb.reshape(GC, 128).T
    bh = b_hh[d][2 * H:].reshape(KC, 128).T                      # (128, KC)
    bhhc = np.repeat(bh[:, :, None], NB, axis=2)                 # (128, KC, NB)
    h0t = h0[d, bsl].reshape(NB, KC, 128).transpose(2, 1, 0)     # (128, KC, NB)
    return {
        "xT": xT_.astype(np.float16),
        "wih": np.ascontiguousarray(wih).astype(np.float16),
        "whh": np.ascontiguousarray(whh).astype(np.float16),
        "biasg": np.ascontiguousarray(biasg).astype(np.float32),
        "ident": np.eye(128, dtype=np.float16),
        "bhhc": np.ascontiguousarray(bhhc).astype(np.float16),
        "h0t": np.ascontiguousarray(h0t).astype(np.float16),
    }


def _assemble(y_cores, t_steps=T):
    y_full = np.zeros((t_steps, B, 2 * H), np.float32)
    hn = np.zeros((2, B, H), np.float32)
    for c in range(N_CORES):
        d, j = divmod(c, 4)
        a = y_cores[c].astype(np.float32)            # (T, 128, KC, NB)
        a = a.transpose(0, 3, 2, 1).reshape(t_steps, NB, H)
        hn[d, j * NB:(j + 1) * NB] = a[-1]
        if d == 1:
            a = a[::-1]
        y_full[:, j * NB:(j + 1) * NB, d * H:(d + 1) * H] = a
    return y_full, hn


_cache = {}


def _get_nc():
    if "nc" not in _cache:
        _cache["nc"] = build()
    return _cache["nc"]


def kernel(x, h0, w_ih, w_hh, b_ih, b_hh, _trace=False):
    from concourse import bass_utils

    x = np.asarray(x, np.float32)
    h0 = np.asarray(h0, np.float32)
    w_ih = np.asarray(w_ih, np.float32)
    w_hh = np.asarray(w_hh, np.float32)
    b_ih = np.asarray(b_ih, np.float32)
    b_hh = np.asarray(b_hh, np.float32)

    nc = _get_nc()
    in_maps = []
    for c in range(N_CORES):
        d, j = divmod(c, 4)
        in_maps.append(_prep_core(x, h0, w_ih, w_hh, b_ih, b_hh, d, j))

    res = bass_utils.run_bass_kernel_spmd(nc, in_maps, list(range(N_CORES)),
                                          trace=_trace)
    y_cores = [res.results[c]["y"] for c in range(N_CORES)]
    y_full, hn = _assemble(y_cores)
    if _trace:
        kernel.last_result = res
    return y_full, hn


# revision 20
# speedup vs baseline: 1.0306x; 1.0306x over previous
"""Bidirectional GRU (T=512, B=32, I=H=512) on 8 Trainium2 NeuronCores.

Sharding: core c -> (direction d = c//4, batch slice j = c%4 of 8).
Backward direction is handled by feeding that core a time-reversed input
and un-reversing its output on the host, so all 8 cores run the same
SPMD program.

Per-core device program:
  Phase 1: xg = x @ w_ih^T + biases as one big fp16 GEMM at full PE
           utilization; xg kept SBUF-resident in fp16 (12 MB).
  Phase 2: 512 sequential GRU steps. Weight-stationary matmuls
           (gate-dim on partitions) so the per-step elementwise work is
           (128, 32)-shaped; h state ping-pongs through 4 fixed SBUF
           slots; per-step y written out by DMA.

Numerics: fp16 operands with fp32 PSUM accumulation and fp32 gate math;
measured end-to-end absmax error vs the fp32 reference ~2.3e-3 on
y (|y|max ~4.1), i.e. ~6e-4 scale-relative.
"""

import numpy as np

T, B, I, H = 512, 32, 512, 512
NB = 8          # batch per core
KC = 4          # contraction chunks (512/128)
GC = 12         # gate-dim chunks (1536/128)
N_CORES = 8
UNROLL = 32
STAGE_AHEAD = 3  # xg prefetch distance (steps)
P1_BLOCK = 512  # phase-1 moving-operand block


def build(t_steps=T, unroll=UNROLL):
    import concourse.bacc as bacc
    import concourse.bass as bass
    import concourse.mybir as mybir
    import concourse.tile as tile
    from contextlib import ExitStack

    f16, f32 = mybir.dt.float16, mybir.dt.float32
    TBS = t_steps * NB

    nc = bacc.Bacc("TRN2", target_bir_lowering=False, debug=False,
                   num_devices=N_CORES)

    xT = nc.dram_tensor("xT", [128, KC, TBS], f16, kind="ExternalInput").ap()
    wih_d = nc.dram_tensor("wih", [128, KC, GC, 128], f16, kind="ExternalInput").ap()
    whh_d = nc.dram_tensor("whh", [128, KC, GC, 128], f16, kind="ExternalInput").ap()
    biasg_d = nc.dram_tensor("biasg", [128, GC], f32, kind="ExternalInput").ap()
    ident_d = nc.dram_tensor("ident", [128, 128], f16, kind="ExternalInput").ap()
    bhhc_d = nc.dram_tensor("bhhc", [128, KC, NB], f16, kind="ExternalInput").ap()
    h0t_d = nc.dram_tensor("h0t", [128, KC, NB], f16, kind="ExternalInput").ap()
    y_d = nc.dram_tensor("y", [t_steps, 128, KC, NB], f16, kind="ExternalOutput").ap()

    ADD = mybir.AluOpType.add
    SUB = mybir.AluOpType.subtract
    SIG = mybir.ActivationFunctionType.Sigmoid
    TANH = mybir.ActivationFunctionType.Tanh

    with ExitStack() as ctx:
        tc = ctx.enter_context(tile.TileContext(nc))
        cp = ctx.enter_context(tc.tile_pool(name="const", bufs=1))
        sp = ctx.enter_context(tc.tile_pool(name="state", bufs=1))
        wp = ctx.enter_context(tc.tile_pool(name="work", bufs=2))

        xts = cp.tile([128, KC, TBS], f16)
        wih_sb = cp.tile([128, KC, GC, 128], f16)
        whh_sb = cp.tile([128, KC, GC, 128], f16)
        bias_sb = cp.tile([128, GC], f32)
        ident_sb = cp.tile([128, 128], f16)
        bhhc_sb = cp.tile([128, KC, NB], f16)
        xg_sb = cp.tile([128, GC, TBS + STAGE_AHEAD * NB], f16)

        nc.sync.dma_start(wih_sb[:], wih_d[:])
        nc.sync.dma_start(whh_sb[:], whh_d[:])
        nc.sync.dma_start(bias_sb[:], biasg_d[:])
        nc.sync.dma_start(ident_sb[:], ident_d[:])
        nc.sync.dma_start(bhhc_sb[:], bhhc_d[:])
        nc.sync.dma_start(xts[:], xT[:])

        # 4 fixed h-state slots; step s reads slot s%4, writes (s+1)%4.
        h_slots = [sp.tile([128, KC, NB], f16, tag=f"h{i}", name=f"h{i}")
                   for i in range(4)]
        nc.sync.dma_start(h_slots[0][:], h0t_d[:])
        # 4 fixed xg staging slots; step s consumes slot s%4 (all-static
        # APs inside the loop), refilled STAGE_AHEAD steps in advance.
        xg_slots = [sp.tile([128, GC, NB], f16, tag=f"xgs{i}", name=f"xgs{i}")
                    for i in range(4)]
        # pad region of xg_sb is read by the tail prefetches; zero it
        nc.vector.memset(xg_sb[:, :, TBS:TBS + STAGE_AHEAD * NB], 0.0)

        # ---- Phase 1: xg[g, t*NB+b] = sum_kc wih[kc,g]^T @ x[kc] + bias[g]
        blk = min(P1_BLOCK, TBS)
        with tc.tile_pool(name="ps1", bufs=4, space="PSUM") as ps1:
            for nb in range(TBS // blk):
                lo, hi = nb * blk, (nb + 1) * blk
                for g in range(GC):
                    ps = ps1.tile([128, blk], f32, tag="p1")
                    for kc in range(KC):
                        nc.tensor.matmul(ps[:], wih_sb[:, kc, g, :],
                                         xts[:, kc, lo:hi],
                                         start=(kc == 0), stop=(kc == KC - 1))
                    nc.vector.tensor_scalar_add(xg_sb[:, g, lo:hi], ps[:],
                                                bias_sb[:, g:g + 1])

        # ---- Phase 2: the recurrence
        # Per-gate psum tiles give precise deps; xg for r/z and b_hh_n are
        # accumulated into PSUM by the PE itself (identity / K=1-row
        # matmuls), so the elementwise chain is:
        #   sig(ps_r) -> *ps_n -> +xg_n -> tanh -> (n-h) -> *zp -> +h
        ps2 = ctx.enter_context(tc.tile_pool(name="ps2", bufs=2, space="PSUM"))
        step_idx = [0]

        # prologue: fill the first STAGE_AHEAD xg staging slots (static)
        for i in range(STAGE_AHEAD):
            nc.sync.dma_start(xg_slots[i][:],
                              xg_sb[:, :, i * NB:(i + 1) * NB])

        def body(iv):
            s = step_idx[0]
            h_prev = h_slots[s % 4]
            h_new = h_slots[(s + 1) % 4]
            xg_t = xg_slots[s % 4]
            step_idx[0] += 1

            # prefetch xg for step iv+STAGE_AHEAD into the slot it will use
            nc.sync.dma_start(
                xg_slots[(s + STAGE_AHEAD) % 4][:],
                xg_sb[:, :, bass.ds((iv + STAGE_AHEAD) * NB, NB)])

            ps_r = ps2.tile([128, KC, NB], f32, tag="ps_r")
            ps_z = ps2.tile([128, KC, NB], f32, tag="ps_z")
            ps_n = ps2.tile([128, KC, NB], f32, tag="ps_n")

            # h-independent psum initializers (identity matmuls) go first:
            # they execute during the PE idle window while this step's h is
            # still being computed. start=True writes xg / b_hh_n into the
            # whole tile; the whh matmuls then accumulate on top
            # (has_written-driven on HW; group checks skipped in sim).
            nc.tensor.matmul(ps_r[:], ident_sb[:], xg_t[:, 0:4, :],
                             start=True, stop=False, skip_group_check=True)
            nc.tensor.matmul(ps_n[:], ident_sb[:], bhhc_sb[:],
                             start=True, stop=False, skip_group_check=True)
            nc.tensor.matmul(ps_z[:], ident_sb[:], xg_t[:, 4:8, :],
                             start=True, stop=False, skip_group_check=True)

            def gate_group(psd, g0):
                for i in range(4):
                    g = g0 + i
                    for kc in range(KC):
                        nc.tensor.matmul(psd[:, i, :], whh_sb[:, kc, g, :],
                                         h_prev[:, kc, :],
                                         start=False,
                                         stop=(i == 3 and kc == KC - 1),
                                         skip_group_check=True)

            gate_group(ps_r, 0)    # r ready earliest
            gate_group(ps_n, 8)    # n next (its chain is longest)
            gate_group(ps_z, 4)    # z last

            r = wp.tile([128, KC, NB], f32, tag="r")
            nc.scalar.activation(r[:], ps_r[:], SIG)
            t1 = wp.tile([128, KC, NB], f32, tag="t1")
            nc.vector.tensor_mul(t1[:], r[:], ps_n[:])
            npre = wp.tile([128, KC, NB], f32, tag="npre")
            nc.vector.tensor_tensor(npre[:], t1[:], xg_t[:, 8:12, :], op=ADD)
            n = wp.tile([128, KC, NB], f32, tag="n")
            tanh_inst = nc.scalar.activation(n[:], npre[:], TANH)
            zp = wp.tile([128, KC, NB], f32, tag="zp")
            zp_inst = nc.scalar.activation(zp[:], ps_z[:], SIG, scale=-1.0)
            tile.add_dep_helper(zp_inst.ins, tanh_inst.ins,
                                reason="keep ACT order: tanh before zp")

            s1 = wp.tile([128, KC, NB], f32, tag="s1")
            nc.vector.tensor_tensor(s1[:], n[:], h_prev[:], op=SUB)
            m = wp.tile([128, KC, NB], f32, tag="m")
            nc.vector.tensor_mul(m[:], zp[:], s1[:])
            nc.vector.tensor_tensor(h_new[:], h_prev[:], m[:], op=ADD)

            nc.sync.dma_start(y_d[bass.ds(iv, 1), :, :, :], h_new[:])

        import concourse.mybir as _mybir

        def unrollable_body(iv0, n_unroll):
            for i in range(n_unroll):
                body(iv0 + i)

        tc.For_i_unrolled_general(0, t_steps, 1, unrollable_body, unroll,
                                  hint_engines=(_mybir.EngineType.PE,))

    nc.compile()
    return nc


def _prep_core(x, h0, w_ih, w_hh, b_ih, b_hh, d, j, t_steps=T):
    bsl = slice(j * NB, (j + 1) * NB)
    xd = x if d == 0 else x[::-1]
    xs = np.ascontiguousarray(xd[:, bsl, :])                     # (T, NB, I)
    xT_ = xs.reshape(t_steps, NB, KC, 128).transpose(3, 2, 0, 1)
    xT_ = np.ascontiguousarray(xT_).reshape(128, KC, t_steps * NB)
    wih = w_ih[d].reshape(GC, 128, KC, 128).transpose(3, 2, 0, 1)
    whh = w_hh[d].reshape(GC, 128, KC, 128).transpose(3, 2, 0, 1)
    bb = b_ih[d].copy()
    bb[:2 * H] += b_hh[d][:2 * H]
    biasg = bb.reshape(GC, 128).T
    bh = b_hh[d][2 * H:].reshape(KC, 128).T                      # (128, KC)
    bhhc = np.repeat(bh[:, :, None], NB, axis=2)                 # (128, KC, NB)
    h0t = h0[d, bsl].reshape(NB, KC, 128).transpose(2, 1, 0)     # (128, KC, NB)
    return {
        "xT": xT_.astype(np.float16),
        "wih": np.ascontiguousarray(wih).astype(np.float16),
        "whh": np.ascontiguousarray(whh).astype(np.float16),
        "biasg": np.ascontiguousarray(biasg).astype(np.float32),
        "ident": np.eye(128, dtype=np.float16),
        "bhhc": np.ascontiguousarray(bhhc).astype(np.float16),
        "h0t": np.ascontiguousarray(h0t).astype(np.float16),
    }


def _assemble(y_cores, t_steps=T):
    y_full = np.zeros((t_steps, B, 2 * H), np.float32)
    hn = np.zeros((2, B, H), np.float32)
    for c in range(N_CORES):
        d, j = divmod(c, 4)
        a = y_cores[c].astype(np.float32)            # (T, 128, KC, NB)
        a = a.transpose(0, 3, 2, 1).reshape(t_steps, NB, H)
        hn[d, j * NB:(j + 1) * NB] = a[-1]
        if d == 1:
            a = a[::-1]
        y_full[:, j * NB:(j + 1) * NB, d * H:(d + 1) * H] = a
    return y_full, hn


_cache = {}


def _get_nc():
    if "nc" not in _cache:
        _cache["nc"] = build()
    return _cache["nc"]


def kernel(x, h0, w_ih, w_hh, b_ih, b_hh, _trace=False):
    from concourse import bass_utils

    x = np.asarray(x, np.float32)
    h0 = np.asarray(h0, np.float32)
    w_ih = np.asarray(w_ih, np.float32)
    w_hh = np.asarray(w_hh, np.float32)
    b_ih = np.asarray(b_ih, np.float32)
    b_hh = np.asarray(b_hh, np.float32)

    nc = _get_nc()
    in_maps = []
    for c in range(N_CORES):
        d, j = divmod(c, 4)
        in_maps.append(_prep_core(x, h0, w_ih, w_hh, b_ih, b_hh, d, j))

    res = bass_utils.run_bass_kernel_spmd(nc, in_maps, list(range(N_CORES)),
                                          trace=_trace)
    y_cores = [res.results[c]["y"] for c in range(N_CORES)]
    y_full, hn = _assemble(y_cores)
    if _trace:
        kernel.last_result = res
    return y_full, hn


# revision 23
# speedup vs baseline: 1.1232x; 1.0899x over previous
"""Bidirectional GRU (T=512, B=32, I=H=512) on 8 Trainium2 NeuronCores.

Sharding: core c -> (direction d = c//4, batch slice j = c%4 of 8).
Backward direction is handled by feeding that core a time-reversed input
and un-reversing its output on the host, so all 8 cores run the same
SPMD program.

Per-core device program:
  Phase 1: xg = x @ w_ih^T + biases as one big fp16 GEMM at full PE
           utilization; xg kept SBUF-resident in fp16 (12 MB).
  Phase 2: 512 sequential GRU steps. Weight-stationary matmuls
           (gate-dim on partitions) so the per-step elementwise work is
           (128, 32)-shaped; h state ping-pongs through 4 fixed SBUF
           slots; per-step y written out by DMA.

Numerics: fp16 operands with fp32 PSUM accumulation and fp32 gate math;
measured end-to-end absmax error vs the fp32 reference ~2.3e-3 on
y (|y|max ~4.1), i.e. ~6e-4 scale-relative.
"""

import numpy as np

T, B, I, H = 512, 32, 512, 512
NB = 8          # batch per core
KC = 4          # contraction chunks (512/128)
GC = 12         # gate-dim chunks (1536/128)
N_CORES = 8
UNROLL = 32
STAGE_W = 4      # steps per xg staging slot
STAGE_PAD = 12   # pad steps at end of xg for tail prefetches
P1_BLOCK = 512  # phase-1 moving-operand block


def build(t_steps=T, unroll=UNROLL):
    import concourse.bacc as bacc
    import concourse.bass as bass
    import concourse.mybir as mybir
    import concourse.tile as tile
    from contextlib import ExitStack

    f16, f32 = mybir.dt.float16, mybir.dt.float32
    TBS = t_steps * NB

    nc = bacc.Bacc("TRN2", target_bir_lowering=False, debug=False,
                   num_devices=N_CORES)

    xT = nc.dram_tensor("xT", [128, KC, TBS], f16, kind="ExternalInput").ap()
    wih_d = nc.dram_tensor("wih", [128, KC, GC, 128], f16, kind="ExternalInput").ap()
    whh_d = nc.dram_tensor("whh", [128, KC, GC, 128], f16, kind="ExternalInput").ap()
    biasg_d = nc.dram_tensor("biasg", [128, GC], f32, kind="ExternalInput").ap()
    ident_d = nc.dram_tensor("ident", [128, 128], f16, kind="ExternalInput").ap()
    bhhc_d = nc.dram_tensor("bhhc", [128, KC, NB], f16, kind="ExternalInput").ap()
    h0t_d = nc.dram_tensor("h0t", [128, KC, NB], f16, kind="ExternalInput").ap()
    y_d = nc.dram_tensor("y", [t_steps, 128, KC, NB], f16, kind="ExternalOutput").ap()

    ADD = mybir.AluOpType.add
    SUB = mybir.AluOpType.subtract
    SIG = mybir.ActivationFunctionType.Sigmoid
    TANH = mybir.ActivationFunctionType.Tanh

    with ExitStack() as ctx:
        tc = ctx.enter_context(tile.TileContext(nc))
        cp = ctx.enter_context(tc.tile_pool(name="const", bufs=1))
        sp = ctx.enter_context(tc.tile_pool(name="state", bufs=1))
        wp = ctx.enter_context(tc.tile_pool(name="work", bufs=2))

        xts = cp.tile([128, KC, TBS], f16)
        wih_sb = cp.tile([128, KC, GC, 128], f16)
        whh_sb = cp.tile([128, KC, GC, 128], f16)
        bias_sb = cp.tile([128, GC], f32)
        ident_sb = cp.tile([128, 128], f16)
        bhhc_sb = cp.tile([128, KC, NB], f16)
        xg_sb = cp.tile([128, GC, TBS + STAGE_PAD * NB], f16)

        nc.sync.dma_start(wih_sb[:], wih_d[:])
        nc.sync.dma_start(whh_sb[:], whh_d[:])
        nc.sync.dma_start(bias_sb[:], biasg_d[:])
        nc.sync.dma_start(ident_sb[:], ident_d[:])
        nc.sync.dma_start(bhhc_sb[:], bhhc_d[:])
        nc.sync.dma_start(xts[:], xT[:])

        # 4 fixed h-state slots; step s reads slot s%4, writes (s+1)%4.
        h_slots = [sp.tile([128, KC, NB], f16, tag=f"h{i}", name=f"h{i}")
                   for i in range(4)]
        nc.sync.dma_start(h_slots[0][:], h0t_d[:])
        # 4 fixed xg staging slots, each holding STAGE_W steps; step s
        # consumes slots[(s//4)%4][:, :, s%4, :] (all-static APs inside the
        # loop); one prefetch DMA per STAGE_W steps, 2 slots ahead.
        xg_slots = [sp.tile([128, GC, STAGE_W, NB], f16,
                            tag=f"xgs{i}", name=f"xgs{i}") for i in range(4)]
        # pad region of xg_sb is read by the tail prefetches; zero it
        nc.vector.memset(xg_sb[:, :, TBS:TBS + STAGE_PAD * NB], 0.0)

        # ---- Phase 1: xg[g, t*NB+b] = sum_kc wih[kc,g]^T @ x[kc] + bias[g]
        blk = min(P1_BLOCK, TBS)
        with tc.tile_pool(name="ps1", bufs=4, space="PSUM") as ps1:
            for nb in range(TBS // blk):
                lo, hi = nb * blk, (nb + 1) * blk
                for g in range(GC):
                    ps = ps1.tile([128, blk], f32, tag="p1")
                    for kc in range(KC):
                        nc.tensor.matmul(ps[:], wih_sb[:, kc, g, :],
                                         xts[:, kc, lo:hi],
                                         start=(kc == 0), stop=(kc == KC - 1))
                    nc.vector.tensor_scalar_add(xg_sb[:, g, lo:hi], ps[:],
                                                bias_sb[:, g:g + 1])

        # ---- Phase 2: the recurrence
        # Per-gate psum tiles give precise deps; xg for r/z and b_hh_n are
        # accumulated into PSUM by the PE itself (identity / K=1-row
        # matmuls), so the elementwise chain is:
        #   sig(ps_r) -> *ps_n -> +xg_n -> tanh -> (n-h) -> *zp -> +h
        ps2 = ctx.enter_context(tc.tile_pool(name="ps2", bufs=2, space="PSUM"))
        step_idx = [0]

        # prologue: fill the first two xg staging slots (static)
        for i in range(2):
            nc.gpsimd.dma_start(
                xg_slots[i][:],
                xg_sb[:, :, i * STAGE_W * NB:(i + 1) * STAGE_W * NB])

        def body(iv):
            s = step_idx[0]
            h_prev = h_slots[s % 4]
            h_new = h_slots[(s + 1) % 4]
            xslot = xg_slots[(s // STAGE_W) % 4]
            w = s % STAGE_W
            step_idx[0] += 1

            # one prefetch per STAGE_W steps, two slots (8 steps) ahead
            if s % STAGE_W == 0:
                nc.gpsimd.dma_start(
                    xg_slots[(s // STAGE_W + 2) % 4][:],
                    xg_sb[:, :, bass.ds((iv + 2 * STAGE_W) * NB,
                                        STAGE_W * NB)])

            ps_r = ps2.tile([128, KC, NB], f32, tag="ps_r")
            ps_z = ps2.tile([128, KC, NB], f32, tag="ps_z")
            ps_n = ps2.tile([128, KC, NB], f32, tag="ps_n")

            # h-independent psum initializers (identity matmuls) go first:
            # they execute during the PE idle window while this step's h is
            # still being computed. start=True writes xg / b_hh_n into the
            # whole tile; the whh matmuls then accumulate on top
            # (has_written-driven on HW; group checks skipped in sim).
            nc.tensor.matmul(ps_r[:], ident_sb[:], xslot[:, 0:4, w, :],
                             start=True, stop=False, skip_group_check=True)
            nc.tensor.matmul(ps_n[:], ident_sb[:], bhhc_sb[:],
                             start=True, stop=False, skip_group_check=True)
            nc.tensor.matmul(ps_z[:], ident_sb[:], xslot[:, 4:8, w, :],
                             start=True, stop=False, skip_group_check=True)

            def gate_group(psd, g0):
                for i in range(4):
                    g = g0 + i
                    for kc in range(KC):
                        nc.tensor.matmul(psd[:, i, :], whh_sb[:, kc, g, :],
                                         h_prev[:, kc, :],
                                         start=False,
                                         stop=(i == 3 and kc == KC - 1),
                                         skip_group_check=True)

            gate_group(ps_r, 0)    # r ready earliest
            gate_group(ps_n, 8)    # n next (its chain is longest)
            gate_group(ps_z, 4)    # z last

            r = wp.tile([128, KC, NB], f32, tag="r")
            nc.scalar.activation(r[:], ps_r[:], SIG)
            t1 = wp.tile([128, KC, NB], f32, tag="t1")
            nc.vector.tensor_mul(t1[:], r[:], ps_n[:])
            npre = wp.tile([128, KC, NB], f32, tag="npre")
            nc.vector.tensor_tensor(npre[:], t1[:], xslot[:, 8:12, w, :], op=ADD)
            n = wp.tile([128, KC, NB], f32, tag="n")
            tanh_inst = nc.scalar.activation(n[:], npre[:], TANH)
            zp = wp.tile([128, KC, NB], f32, tag="zp")
            zp_inst = nc.scalar.activation(zp[:], ps_z[:], SIG, scale=-1.0)
            tile.add_dep_helper(zp_inst.ins, tanh_inst.ins,
                                reason="keep ACT order: tanh before zp")

            s1 = wp.tile([128, KC, NB], f32, tag="s1")
            nc.vector.tensor_tensor(s1[:], n[:], h_prev[:], op=SUB)
            m = wp.tile([128, KC, NB], f32, tag="m")
            nc.vector.tensor_mul(m[:], zp[:], s1[:])
            nc.vector.tensor_tensor(h_new[:], h_prev[:], m[:], op=ADD)

            nc.sync.dma_start(y_d[bass.ds(iv, 1), :, :, :], h_new[:])

        import concourse.mybir as _mybir

        def unrollable_body(iv0, n_unroll):
            for i in range(n_unroll):
                body(iv0 + i)

        tc.For_i_unrolled_general(0, t_steps, 1, unrollable_body, unroll,
                                  hint_engines=(_mybir.EngineType.PE,))

    nc.compile()
    return nc


def _prep_core(x, h0, w_ih, w_hh, b_ih, b_hh, d, j, t_steps=T):
    bsl = slice(j * NB, (j + 1) * NB)
    xd = x if d == 0 else x[::-1]
    xs = np.ascontiguousarray(xd[:, bsl, :])                     # (T, NB, I)
    xT_ = xs.reshape(t_steps, NB, KC, 128).transpose(3, 2, 0, 1)
    xT_ = np.ascontiguousarray(xT_).reshape(128, KC, t_steps * NB)
    wih = w_ih[d].reshape(GC, 128, KC, 128).transpose(3, 2, 0, 1)
    whh = w_hh[d].reshape(GC, 128, KC, 128).transpose(3, 2, 0, 1)
    bb = b_ih[d].copy()
    bb[:2 * H] += b_hh[d][:2 * H]
    biasg = bb.reshape(GC, 128).T
    bh = b_hh[d][2 * H:].reshape(KC, 128).T                      # (128, KC)
    bhhc = np.repeat(bh[:, :, None], NB, axis=2)                 # (128, KC, NB)
    h0t = h0[d, bsl].reshape(NB, KC, 128).transpose(2, 1, 0)     # (128, KC, NB)
    return {
        "xT": xT_.astype(np.float16),
        "wih": np.ascontiguousarray(wih).astype(np.float16),
        "whh": np.ascontiguousarray(whh).astype(np.float16),
        "biasg": np.ascontiguousarray(biasg).astype(np.float32),
        "ident": np.eye(128, dtype=np.float16),
        "bhhc": np.ascontiguousarray(bhhc).astype(np.float16),
        "h0t": np.ascontiguousarray(h0t).astype(np.float16),
    }


def _assemble(y_cores, t_steps=T):
    y_full = np.zeros((t_steps, B, 2 * H), np.float32)
    hn = np.zeros((2, B, H), np.float32)
    for c in range(N_CORES):
        d, j = divmod(c, 4)
        a = y_cores[c].astype(np.float32)            # (T, 128, KC, NB)
        a = a.transpose(0, 3, 2, 1).reshape(t_steps, NB, H)
        hn[d, j * NB:(j + 1) * NB] = a[-1]
        if d == 1:
            a = a[::-1]
        y_full[:, j * NB:(j + 1) * NB, d * H:(d + 1) * H] = a
    return y_full, hn


_cache = {}


def _get_nc():
    if "nc" not in _cache:
        _cache["nc"] = build()
    return _cache["nc"]


def kernel(x, h0, w_ih, w_hh, b_ih, b_hh, _trace=False):
    from concourse import bass_utils

    x = np.asarray(x, np.float32)
    h0 = np.asarray(h0, np.float32)
    w_ih = np.asarray(w_ih, np.float32)
    w_hh = np.asarray(w_hh, np.float32)
    b_ih = np.asarray(b_ih, np.float32)
    b_hh = np.asarray(b_hh, np.float32)

    nc = _get_nc()
    in_maps = []
    for c in range(N_CORES):
        d, j = divmod(c, 4)
        in_maps.append(_prep_core(x, h0, w_ih, w_hh, b_ih, b_hh, d, j))

    res = bass_utils.run_bass_kernel_spmd(nc, in_maps, list(range(N_CORES)),
                                          trace=_trace)
    y_cores = [res.results[c]["y"] for c in range(N_CORES)]
    y_full, hn = _assemble(y_cores)
    if _trace:
        kernel.last_result = res
    return y_full, hn


# revision 25
# speedup vs baseline: 1.1407x; 1.0156x over previous
"""Bidirectional GRU (T=512, B=32, I=H=512) on 8 Trainium2 NeuronCores.

Sharding: core c -> (direction d = c//4, batch slice j = c%4 of 8).
Backward direction is handled by feeding that core a time-reversed input
and un-reversing its output on the host, so all 8 cores run the same
SPMD program.

Per-core device program:
  Phase 1: xg = x @ w_ih^T + biases as one big fp16 GEMM at full PE
           utilization; xg kept SBUF-resident in fp16 (12 MB).
  Phase 2: 512 sequential GRU steps. Weight-stationary matmuls
           (gate-dim on partitions) so the per-step elementwise work is
           (128, 32)-shaped; h state ping-pongs through 4 fixed SBUF
           slots; per-step y written out by DMA.

Numerics: fp16 operands with fp32 PSUM accumulation and fp32 gate math;
measured end-to-end absmax error vs the fp32 reference ~2.3e-3 on
y (|y|max ~4.1), i.e. ~6e-4 scale-relative.
"""

import numpy as np

T, B, I, H = 512, 32, 512, 512
NB = 8          # batch per core
KC = 4          # contraction chunks (512/128)
GC = 12         # gate-dim chunks (1536/128)
N_CORES = 8
UNROLL = 32
STAGE_W = 4      # steps per xg staging slot
STAGE_PAD = 12   # pad steps at end of xg for tail prefetches
P1_BLOCK = 512  # phase-1 moving-operand block
STATIC_PREFIX = 96  # statically-unrolled first steps (phase-1 overlap)


def build(t_steps=T, unroll=UNROLL):
    import concourse.bacc as bacc
    import concourse.bass as bass
    import concourse.mybir as mybir
    import concourse.tile as tile
    from contextlib import ExitStack

    f16, f32 = mybir.dt.float16, mybir.dt.float32
    TBS = t_steps * NB

    nc = bacc.Bacc("TRN2", target_bir_lowering=False, debug=False,
                   num_devices=N_CORES)

    xT = nc.dram_tensor("xT", [128, KC, TBS], f16, kind="ExternalInput").ap()
    wih_d = nc.dram_tensor("wih", [128, KC, GC, 128], f16, kind="ExternalInput").ap()
    whh_d = nc.dram_tensor("whh", [128, KC, GC, 128], f16, kind="ExternalInput").ap()
    biasg_d = nc.dram_tensor("biasg", [128, GC], f32, kind="ExternalInput").ap()
    ident_d = nc.dram_tensor("ident", [128, 128], f16, kind="ExternalInput").ap()
    bhhc_d = nc.dram_tensor("bhhc", [128, KC, NB], f16, kind="ExternalInput").ap()
    h0t_d = nc.dram_tensor("h0t", [128, KC, NB], f16, kind="ExternalInput").ap()
    y_d = nc.dram_tensor("y", [t_steps, 128, KC, NB], f16, kind="ExternalOutput").ap()

    ADD = mybir.AluOpType.add
    SUB = mybir.AluOpType.subtract
    SIG = mybir.ActivationFunctionType.Sigmoid
    TANH = mybir.ActivationFunctionType.Tanh

    with ExitStack() as ctx:
        tc = ctx.enter_context(tile.TileContext(nc))
        cp = ctx.enter_context(tc.tile_pool(name="const", bufs=1))
        sp = ctx.enter_context(tc.tile_pool(name="state", bufs=1))
        wp = ctx.enter_context(tc.tile_pool(name="work", bufs=2))

        xts = cp.tile([128, KC, TBS], f16)
        wih_sb = cp.tile([128, KC, GC, 128], f16)
        whh_sb = cp.tile([128, KC, GC, 128], f16)
        bias_sb = cp.tile([128, GC], f32)
        ident_sb = cp.tile([128, 128], f16)
        bhhc_sb = cp.tile([128, KC, NB], f16)
        xg_sb = cp.tile([128, GC, TBS + STAGE_PAD * NB], f16)

        nc.sync.dma_start(wih_sb[:], wih_d[:])
        nc.sync.dma_start(whh_sb[:], whh_d[:])
        nc.sync.dma_start(bias_sb[:], biasg_d[:])
        nc.sync.dma_start(ident_sb[:], ident_d[:])
        nc.sync.dma_start(bhhc_sb[:], bhhc_d[:])
        nc.sync.dma_start(xts[:], xT[:])

        # 4 fixed h-state slots; step s reads slot s%4, writes (s+1)%4.
        h_slots = [sp.tile([128, KC, NB], f16, tag=f"h{i}", name=f"h{i}")
                   for i in range(4)]
        nc.sync.dma_start(h_slots[0][:], h0t_d[:])
        # 4 fixed xg staging slots, each holding STAGE_W steps; step s
        # consumes slots[(s//4)%4][:, :, s%4, :] (all-static APs inside the
        # loop); one prefetch DMA per STAGE_W steps, 2 slots ahead.
        xg_slots = [sp.tile([128, GC, STAGE_W, NB], f16,
                            tag=f"xgs{i}", name=f"xgs{i}") for i in range(4)]
        # pad region of xg_sb is read by the tail prefetches; zero it
        nc.vector.memset(xg_sb[:, :, TBS:TBS + STAGE_PAD * NB], 0.0)

        # ---- Phase 1: xg[g, t*NB+b] = sum_kc wih[kc,g]^T @ x[kc] + bias[g]
        # With a static prefix, only the first 2 blocks run up front; the
        # rest are emitted interleaved with the first recurrence steps and
        # fill the PE idle windows there.
        blk = min(P1_BLOCK, TBS)
        nblocks = TBS // blk
        prefix = STATIC_PREFIX if t_steps >= 2 * STATIC_PREFIX else 0
        ps1 = ctx.enter_context(tc.tile_pool(name="ps1", bufs=2, space="PSUM"))

        def emit_block(nb):
            lo, hi = nb * blk, (nb + 1) * blk
            for g in range(GC):
                ps = ps1.tile([128, blk], f32, tag="p1", name="p1ps")
                for kc in range(KC):
                    nc.tensor.matmul(ps[:], wih_sb[:, kc, g, :],
                                     xts[:, kc, lo:hi],
                                     start=(kc == 0), stop=(kc == KC - 1))
                nc.vector.tensor_scalar_add(xg_sb[:, g, lo:hi], ps[:],
                                            bias_sb[:, g:g + 1])

        upfront = nblocks if prefix == 0 else min(2, nblocks)
        for nb in range(upfront):
            emit_block(nb)

        # ---- Phase 2: the recurrence
        # Per-gate psum tiles give precise deps; xg for r/z and b_hh_n are
        # accumulated into PSUM by the PE itself (identity / K=1-row
        # matmuls), so the elementwise chain is:
        #   sig(ps_r) -> *ps_n -> +xg_n -> tanh -> (n-h) -> *zp -> +h
        ps2 = ctx.enter_context(tc.tile_pool(name="ps2", bufs=2, space="PSUM"))
        step_idx = [0]


        def body(iv, static_xg=False):
            s = step_idx[0]
            h_prev = h_slots[s % 4]
            h_new = h_slots[(s + 1) % 4]
            step_idx[0] += 1

            if static_xg:
                xg_r = xg_sb[:, 0:4, iv * NB:(iv + 1) * NB]
                xg_z = xg_sb[:, 4:8, iv * NB:(iv + 1) * NB]
                xg_n = xg_sb[:, 8:12, iv * NB:(iv + 1) * NB]
            else:
                xslot = xg_slots[(s // STAGE_W) % 4]
                w = s % STAGE_W
                xg_r = xslot[:, 0:4, w, :]
                xg_z = xslot[:, 4:8, w, :]
                xg_n = xslot[:, 8:12, w, :]
                # one prefetch per STAGE_W steps, two slots (8 steps) ahead
                if s % STAGE_W == 0:
                    nc.gpsimd.dma_start(
                        xg_slots[(s // STAGE_W + 2) % 4][:],
                        xg_sb[:, :, bass.ds((iv + 2 * STAGE_W) * NB,
                                            STAGE_W * NB)])

            ps_r = ps2.tile([128, KC, NB], f32, tag="ps_r")
            ps_z = ps2.tile([128, KC, NB], f32, tag="ps_z")
            ps_n = ps2.tile([128, KC, NB], f32, tag="ps_n")

            # h-independent psum initializers (identity matmuls) go first:
            # they execute during the PE idle window while this step's h is
            # still being computed. start=True writes xg / b_hh_n into the
            # whole tile; the whh matmuls then accumulate on top
            # (has_written-driven on HW; group checks skipped in sim).
            nc.tensor.matmul(ps_r[:], ident_sb[:], xg_r,
                             start=True, stop=False, skip_group_check=True)
            nc.tensor.matmul(ps_n[:], ident_sb[:], bhhc_sb[:],
                             start=True, stop=False, skip_group_check=True)
            nc.tensor.matmul(ps_z[:], ident_sb[:], xg_z,
                             start=True, stop=False, skip_group_check=True)

            def gate_group(psd, g0):
                for i in range(4):
                    g = g0 + i
                    for kc in range(KC):
                        nc.tensor.matmul(psd[:, i, :], whh_sb[:, kc, g, :],
                                         h_prev[:, kc, :],
                                         start=False,
                                         stop=(i == 3 and kc == KC - 1),
                                         skip_group_check=True)

            gate_group(ps_r, 0)    # r ready earliest
            gate_group(ps_n, 8)    # n next (its chain is longest)
            gate_group(ps_z, 4)    # z last

            r = wp.tile([128, KC, NB], f32, tag="r")
            nc.scalar.activation(r[:], ps_r[:], SIG)
            t1 = wp.tile([128, KC, NB], f32, tag="t1")
            nc.vector.tensor_mul(t1[:], r[:], ps_n[:])
            npre = wp.tile([128, KC, NB], f32, tag="npre")
            nc.vector.tensor_tensor(npre[:], t1[:], xg_n, op=ADD)
            n = wp.tile([128, KC, NB], f32, tag="n")
            tanh_inst = nc.scalar.activation(n[:], npre[:], TANH)
            zp = wp.tile([128, KC, NB], f32, tag="zp")
            zp_inst = nc.scalar.activation(zp[:], ps_z[:], SIG, scale=-1.0)
            tile.add_dep_helper(zp_inst.ins, tanh_inst.ins,
                                reason="keep ACT order: tanh before zp")

            s1 = wp.tile([128, KC, NB], f32, tag="s1")
            nc.vector.tensor_tensor(s1[:], n[:], h_prev[:], op=SUB)
            m = wp.tile([128, KC, NB], f32, tag="m")
            nc.vector.tensor_mul(m[:], zp[:], s1[:])
            nc.vector.tensor_tensor(h_new[:], h_prev[:], m[:], op=ADD)

            nc.sync.dma_start(y_d[bass.ds(iv, 1), :, :, :], h_new[:])

        import concourse.mybir as _mybir

        # static prefix: phase-1 blocks 2.. are emitted between these steps
        # and get scheduled into the PE idle windows
        for t in range(prefix):
            if t % 16 == 0 and 2 + t // 16 < nblocks:
                emit_block(2 + t // 16)
            body(t, static_xg=True)

        # staging-slot prologue for the dynamic loop's first 8 steps
        p0 = (prefix // STAGE_W) % 4
        for i in range(2):
            nc.gpsimd.dma_start(
                xg_slots[(p0 + i) % 4][:],
                xg_sb[:, :, (prefix + i * STAGE_W) * NB:
                            (prefix + (i + 1) * STAGE_W) * NB])

        def unrollable_body(iv0, n_unroll):
            for i in range(n_unroll):
                body(iv0 + i)

        tc.For_i_unrolled_general(prefix, t_steps, 1, unrollable_body, unroll,
                                  hint_engines=(_mybir.EngineType.PE,))

    nc.compile()
    return nc


def _prep_core(x, h0, w_ih, w_hh, b_ih, b_hh, d, j, t_steps=T):
    bsl = slice(j * NB, (j + 1) * NB)
    xd = x if d == 0 else x[::-1]
    xs = np.ascontiguousarray(xd[:, bsl, :])                     # (T, NB, I)
    xT_ = xs.reshape(t_steps, NB, KC, 128).transpose(3, 2, 0, 1)
    xT_ = np.ascontiguousarray(xT_).reshape(128, KC, t_steps * NB)
    wih = w_ih[d].reshape(GC, 128, KC, 128).transpose(3, 2, 0, 1)
    whh = w_hh[d].reshape(GC, 128, KC, 128).transpose(3, 2, 0, 1)
    bb = b_ih[d].copy()
    bb[:2 * H] += b_hh[d][:2 * H]
    biasg = bb.reshape(GC, 128).T
    bh = b_hh[d][2 * H:].reshape(KC, 128).T                      # (128, KC)
    bhhc = np.repeat(bh[:, :, None], NB, axis=2)                 # (128, KC, NB)
    h0t = h0[d, bsl].reshape(NB, KC, 128).transpose(2, 1, 0)     # (128, KC, NB)
    return {
        "xT": xT_.astype(np.float16),
        "wih": np.ascontiguousarray(wih).astype(np.float16),
        "whh": np.ascontiguousarray(whh).astype(np.float16),
        "biasg": np.ascontiguousarray(biasg).astype(np.float32),
        "ident": np.eye(128, dtype=np.float16),
        "bhhc": np.ascontiguousarray(bhhc).astype(np.float16),
        "h0t": np.ascontiguousarray(h0t).astype(np.float16),
    }


def _assemble(y_cores, t_steps=T):
    y_full = np.zeros((t_steps, B, 2 * H), np.float32)
    hn = np.zeros((2, B, H), np.float32)
    for c in range(N_CORES):
        d, j = divmod(c, 4)
        a = y_cores[c].astype(np.float32)            # (T, 128, KC, NB)
        a = a.transpose(0, 3, 2, 1).reshape(t_steps, NB, H)
        hn[d, j * NB:(j + 1) * NB] = a[-1]
        if d == 1:
            a = a[::-1]
        y_full[:, j * NB:(j + 1) * NB, d * H:(d + 1) * H] = a
    return y_full, hn


_cache = {}


def _get_nc():
    if "nc" not in _cache:
        _cache["nc"] = build()
    return _cache["nc"]


def kernel(x, h0, w_ih, w_hh, b_ih, b_hh, _trace=False):
    from concourse import bass_utils

    x = np.asarray(x, np.float32)
    h0 = np.asarray(h0, np.float32)
    w_ih = np.asarray(w_ih, np.float32)
    w_hh = np.asarray(w_hh, np.float32)
    b_ih = np.asarray(b_ih, np.float32)
    b_hh = np.asarray(b_hh, np.float32)

    nc = _get_nc()
    in_maps = []
    for c in range(N_CORES):
        d, j = divmod(c, 4)
        in_maps.append(_prep_core(x, h0, w_ih, w_hh, b_ih, b_hh, d, j))

    res = bass_utils.run_bass_kernel_spmd(nc, in_maps, list(range(N_CORES)),
                                          trace=_trace)
    y_cores = [res.results[c]["y"] for c in range(N_CORES)]
    y_full, hn = _assemble(y_cores)
    if _trace:
        kernel.last_result = res
    return y_full, hn
